# revision 29
# baseline (speedup 1.0000x reference)
"""CategorySpecificLinear Trainium2 kernel.

out[t] = x[t] @ weight[category_id[t]] + bias[category_id[t]]

Strategy: expert-parallel over the 8 categories (C == n_cores == 8).
The host routes tokens by category, transposes each category's token
block to [D, T_pad], converts x and w to bf16, and hands core c:
    xT   [D, T_pad]   bf16 tokens of category c, zero-padded to T_pad
    w    [D, O]       bf16 weight[c]
    bias [1, O]       f32 bias[c]
Each core computes out = xT.T @ w + bias with bf16 N=512 matmuls
(1 col/cycle at 2.4 GHz warm = 216 ns/MM, ~2x the fp32r rate; psum
accumulates in f32 so the only loss is bf16 input/output rounding,
rel err ~3e-3 vs the 2e-2 gate). Output is written bf16 and upcast on
the host. The 80-MM stream (~17.3 us) is the per-core compute
roofline: resharding cannot shrink it, so the schedule starts it as
early as the ~7.2 us framework preamble allows and keeps it gapless:
  - every m-tile is a full 128 wide (a 32-row remainder tile stalls
    the LDWEIGHTS pipeline ~2x ~100 ns per k-step, measured); the pad
    lhsT columns are junk SBUF that only feeds never-read psum rows
  - wave 0 (n=0) is k-lockstep, fed round-robin by all three DMA
    queues (~110 GB/s each); wave 1 is k-contiguous per m-tile so
    each psum retires early and its DVE+out drain overlaps the
    remaining matmuls
  - ~2.4 us of dummy matmuls bridge the preamble->first-data gap so
    the HAM clock gate (4/8 -> 8/8 after ~3.4 us of sustained PE
    activity) opens just as the real stream starts
Measured: 34.6-35.5 us NEFF exec (vs 42.5-43.3 us fp32r baseline),
of which ~7.2 us fixed preamble + ~3 us fixed epilogue; rel err 2.9e-3.
"""

import contextlib
import ctypes
import os
import sys
import types

import ml_dtypes
import numpy as np

BF16 = np.dtype(ml_dtypes.bfloat16)

sys.path.insert(0, "/opt/trn_rl_repo")


def _ensure_ntff_hook():
    """Provide antenv.axon_hooks if the image lacks it.

    concourse.bass_utils imports antenv.axon_hooks.get_axon_ntff_profile_hook
    when trace=True under axon; some agent images don't ship that module, in
    which case the boot's NTFF hook registration silently degrades and the
    import in bass_utils crashes. Recreate the slim ctypes hook here
    (mirrors trn_agent_boot.trn_boot._ntff_profile_via_ctypes).
    """
    try:
        import antenv.axon_hooks  # noqa: F401

        return
    except ImportError:
        pass

    so_path = "/opt/axon/libaxon_pjrt.so"
    hook = None
    if os.path.exists(so_path):
        lib = ctypes.CDLL(so_path)
        if hasattr(lib, "axon_start_nrt_profile"):
            lib.axon_start_nrt_profile.argtypes = [
                ctypes.POINTER(ctypes.c_int64),
                ctypes.c_size_t,
            ]
            lib.axon_start_nrt_profile.restype = ctypes.c_int64
            lib.axon_stop_nrt_profile.argtypes = [ctypes.c_char_p]
            lib.axon_stop_nrt_profile.restype = ctypes.c_int64

            @contextlib.contextmanager
            def hook(output_dir, device_ids):
                import jax

                jax.devices()
                if device_ids:
                    ids = (ctypes.c_int64 * len(device_ids))(*device_ids)
                    rc = lib.axon_start_nrt_profile(ids, len(device_ids))
                else:
                    rc = lib.axon_start_nrt_profile(None, 0)
                if rc != 0:
                    raise RuntimeError(f"axon_start_nrt_profile rc={rc}")
                try:
                    yield
                finally:
                    n = lib.axon_stop_nrt_profile(str(output_dir).encode())
                    if n <= 0:
                        print(
                            f"ntff profile: rc={n} writing {output_dir}",
                            file=sys.stderr,
                        )

    mod = types.ModuleType("antenv.axon_hooks")
    _state = {"hook": hook}
    mod.set_axon_ntff_profile_hook = lambda h: _state.__setitem__("hook", h)
    mod.get_axon_ntff_profile_hook = lambda: _state["hook"]
    sys.modules["antenv.axon_hooks"] = mod
    try:
        import antenv

        antenv.axon_hooks = mod
    except ImportError:
        pass


_ensure_ntff_hook()

import concourse.bass as bass
import concourse.bacc as bacc_mod
import concourse.mybir as mybir
import concourse.tile as tile
from concourse.bass import ts
from concourse.bass_utils import run_bass_kernel_spmd

N_CORES = 8
P = 128
N_TILE = 512  # one fp32 PSUM bank; also >=256 keeps fp32r at full rate

_nc_cache = {}
LAST_RESULTS = None  # BassKernelResults of the most recent run (for test.py)


N_WARM = 22  # HAM warm-up matmuls issued before the real stream


def _build_nc(T_use: int, D: int, O: int):
    KO = D // P
    NO = O // N_TILE
    mmdt = mybir.dt.bfloat16
    f32 = mybir.dt.float32

    # All matmul m-tiles are a full 128 wide: a short (e.g. 32-row)
    # remainder tile in the k-lockstep group stalls the LDWEIGHTS
    # pipeline twice per k-step (~+210 ns/step, measured). The pad
    # columns of x are never DMA'd (junk SBUF is fine: they only feed
    # psum rows that are never read) and the pad rows of out are never
    # written; only T_use rows are real.
    T_pad = -(-T_use // P) * P
    MO = T_pad // P
    m_starts = [P * i for i in range(MO)]
    o_sizes = [min(P, T_use - s) for s in m_starts]  # DVE/out rows per tile

    nc = bacc_mod.Bacc()
    xT = nc.dram_tensor("xT", [D, T_use], mmdt, kind="ExternalInput")
    w = nc.dram_tensor("w", [D, O], mmdt, kind="ExternalInput")
    bias = nc.dram_tensor("bias", [1, O], f32, kind="ExternalInput")
    out = nc.dram_tensor("out", [T_use, O], mmdt, kind="ExternalOutput")

    xT_t = xT[:, :].rearrange("(ko p) t -> p ko t", p=P)
    w_t = w[:, :].rearrange("(ko p) o -> p ko o", p=P)

    with tile.TileContext(nc) as tc:
        with (
            tc.tile_pool(name="resident", bufs=1) as rpool,
            tc.tile_pool(name="psum", bufs=7, space="PSUM") as psum_pool,
            tc.tile_pool(name="warmps", bufs=1, space="PSUM") as warm_pool,
            tc.tile_pool(name="obuf", bufs=6) as opool,
        ):
            # HAM warm-up: the PE clock gate needs ~3.4 us of sustained
            # activity to go 4/8 -> 8/8; dummy matmuls bridge the gap
            # between the framework preamble end (~7.2 us) and the first
            # real data landing (~8.5 us). The dummy psum is never read.
            warm_sb = rpool.tile([P, 128], mmdt, tag="warm")
            nc.vector.memset(warm_sb[:], 0.0)
            warm_ps = warm_pool.tile([64, 128], f32, tag="wps")
            for i in range(N_WARM):
                nc.tensor.matmul(
                    warm_ps[:],
                    lhsT=warm_sb[:, :64],
                    rhs=warm_sb[:],
                    start=True,
                    stop=True,
                )
            # Each DMA queue sustains only ~110 GB/s (~90 on the SWDGE/
            # gpsimd queue), and wave 0 consumes x(k)+w(k,0) at
            # ~250 GB/s, so ALL THREE queues carry wave 0's pieces,
            # round-robin in k order (earliest-needed first). Wave 1's
            # w(:,1) follows on the same rotation, and the bias halves
            # ride behind (the DVE first needs bias-n0 only when wave 0
            # drains, ~10 us after the stream starts).
            bias_sb = rpool.tile([P, O], f32, tag="bias")
            x_sb = {}
            w_sb = {}
            rr = [nc.sync, nc.scalar, nc.gpsimd]
            rri = 0

            def next_eng():
                nonlocal rri
                eng = rr[rri % 3]
                rri += 1
                return eng

            def load_x(k):
                xt = rpool.tile([P, T_pad], mmdt, tag=f"x{k}", name=f"x{k}")
                next_eng().dma_start(xt[:, :T_use], xT_t[:, k, :])
                x_sb[k] = xt

            def load_w(k, n):
                wt = rpool.tile([P, N_TILE], mmdt, tag=f"w{k}_{n}", name=f"w{k}_{n}")
                next_eng().dma_start(wt[:], w_t[:, k, ts(n, N_TILE)])
                w_sb[(k, n)] = wt

            # k=0's pair gates the stream start: split each piece into
            # two half-DMAs on one queue so the second issue's ~0.65 us
            # engine cost overlaps the first half's transfer (~0.6 us
            # earlier first-data, measured ~10.0 -> ~9.4 us).
            wt = rpool.tile([P, N_TILE], mmdt, tag="w0_0", name="w0_0")
            half = N_TILE // 2
            nc.sync.dma_start(wt[:, :half], w_t[:, 0, 0:half])
            nc.sync.dma_start(wt[:, half:], w_t[:, 0, half:N_TILE])
            w_sb[(0, 0)] = wt
            xt = rpool.tile([P, T_pad], mmdt, tag="x0", name="x0")
            xh = T_use // 2
            nc.scalar.dma_start(xt[:, :xh], xT_t[:, 0, :xh])
            nc.scalar.dma_start(xt[:, xh:T_use], xT_t[:, 0, xh:])
            x_sb[0] = xt
            rri = 2  # resume the rotation at gpsimd for w(1,0)
            for k in range(1, KO):
                load_w(k, 0)
                load_x(k)
                if k >= 5:  # wave-1 weights start landing before wave 0 ends
                    load_w(k - 5, 1)
            for k in range(3, KO):
                load_w(k, 1)
            nc.sync.dma_start(
                bias_sb[:, ts(0, N_TILE)],
                bias[:, ts(0, N_TILE)].to_broadcast((P, N_TILE)),
            )
            nc.scalar.dma_start(
                bias_sb[:, ts(1, N_TILE)],
                bias[:, ts(1, N_TILE)].to_broadcast((P, N_TILE)),
            )

            # One wave per n-tile: all MO psum groups accumulate in lockstep
            # over k, so the k-th step only needs x(k)/w(k,n). Output DMAs
            # alternate sync/scalar (free once inputs land); the final
            # three spread over all queues so the tail never serializes
            # behind one ~110 GB/s queue.
            n_outs = NO * MO
            out_engs = ([nc.sync, nc.scalar] * n_outs)[:n_outs]
            out_engs[n_outs - 3 : n_outs] = [nc.gpsimd, nc.sync, nc.scalar]
            def drain(n, m, ps_):
                ot = opool.tile([P, N_TILE], mmdt, tag="ot", name=f"ot{n}_{m}")
                nc.vector.tensor_add(
                    ot[: o_sizes[m]],
                    ps_[: o_sizes[m]],
                    bias_sb[: o_sizes[m], ts(n, N_TILE)],
                )
                out_engs[n * MO + m].dma_start(
                    out[m_starts[m] : m_starts[m] + o_sizes[m], ts(n, N_TILE)],
                    ot[: o_sizes[m]],
                )

            # Wave 0 is k-lockstep (x/w arrive k-major, the PE starts
            # after the first pair). Wave 1 has everything resident, so
            # it goes k-contiguous per m-tile: each tile's psum retires
            # ~1.7 us apart and its DVE+out drain hides under the
            # remaining matmuls — only the last tile's drain is tail.
            pss = [
                psum_pool.tile([P, N_TILE], f32, tag="ps", name=f"ps0_{m}")
                for m in range(MO)
            ]
            for k in range(KO):
                for m in range(MO):
                    nc.tensor.matmul(
                        pss[m][:],
                        lhsT=x_sb[k][:, m_starts[m] : m_starts[m] + P],
                        rhs=w_sb[(k, 0)][:],
                        start=(k == 0),
                        stop=(k == KO - 1),
                    )
            for m in range(MO):
                drain(0, m, pss[m])
            for m in range(MO):
                ps_ = psum_pool.tile([P, N_TILE], f32, tag="ps", name=f"ps1_{m}")
                for k in range(KO):
                    nc.tensor.matmul(
                        ps_[:],
                        lhsT=x_sb[k][:, m_starts[m] : m_starts[m] + P],
                        rhs=w_sb[(k, 1)][:],
                        start=(k == 0),
                        stop=(k == KO - 1),
                    )
                drain(1, m, ps_)
    nc.finalize()
    return nc


def kernel(x, category_id, weight, bias):
    global LAST_RESULTS
    x = np.asarray(x)
    category_id = np.asarray(category_id)
    weight = np.ascontiguousarray(np.asarray(weight), dtype=np.float32)
    bias = np.ascontiguousarray(np.asarray(bias), dtype=np.float32)

    orig_shape = x.shape
    D = orig_shape[-1]
    C, _, O = weight.shape
    assert C == N_CORES and D % P == 0 and O % N_TILE == 0

    T = int(np.prod(orig_shape[:-1]))
    x_flat = np.ascontiguousarray(x.reshape(T, D), dtype=np.float32)
    cid = category_id.reshape(T).astype(np.int64)

    idx_per_c = [np.flatnonzero(cid == c) for c in range(C)]
    counts = [len(ix) for ix in idx_per_c]
    T_pad = max(32, -(-max(counts) // 32) * 32)  # multiple of 32 (PE col-group)

    key = (T_pad, D, O)
    if key not in _nc_cache:
        _nc_cache[key] = _build_nc(T_pad, D, O)
    nc = _nc_cache[key]

    in_maps = []
    for c in range(C):
        xcT = np.zeros((D, T_pad), dtype=BF16)
        xcT[:, : counts[c]] = x_flat[idx_per_c[c]].astype(BF16).T
        in_maps.append(
            {
                "xT": xcT,
                "w": weight[c].astype(BF16),
                "bias": bias[c : c + 1],
            }
        )

    res = run_bass_kernel_spmd(nc, in_maps, list(range(N_CORES)))
    LAST_RESULTS = res

    out_flat = np.empty((T, O), dtype=np.float32)
    for c in range(C):
        out_flat[idx_per_c[c]] = res.results[c]["out"][: counts[c]].astype(np.float32)
    return out_flat.reshape(*orig_shape[:-1], O)



# revision 30
# speedup vs baseline: 1.0113x; 1.0113x over previous
"""CategorySpecificLinear Trainium2 kernel.

out[t] = x[t] @ weight[category_id[t]] + bias[category_id[t]]

Strategy: expert-parallel over the 8 categories (C == n_cores == 8).
The host routes tokens by category, transposes each category's token
block to [D, T_pad], converts x and w to bf16, and hands core c:
    xT   [D, T_pad]   bf16 tokens of category c, zero-padded to T_pad
    w    [D, O]       bf16 weight[c]
    bias [1, O]       f32 bias[c]
Each core computes out = xT.T @ w + bias with bf16 N=512 matmuls
(1 col/cycle at 2.4 GHz warm = 216 ns/MM, ~2x the fp32r rate; psum
accumulates in f32 so the only loss is bf16 input/output rounding,
rel err ~3e-3 vs the 2e-2 gate). Output is written bf16 and upcast on
the host. The 80-MM stream (~17.3 us) is the per-core compute
roofline: resharding cannot shrink it, so the schedule starts it as
early as the ~7.2 us framework preamble allows and keeps it gapless:
  - every m-tile is a full 128 wide (a 32-row remainder tile stalls
    the LDWEIGHTS pipeline ~2x ~100 ns per k-step, measured); the pad
    lhsT columns are junk SBUF that only feeds never-read psum rows
  - wave 0 (n=0) is k-lockstep, fed round-robin by all three DMA
    queues (~110 GB/s each); wave 1 is k-contiguous per m-tile so
    each psum retires early and its DVE+out drain overlaps the
    remaining matmuls
  - ~2.4 us of dummy matmuls bridge the preamble->first-data gap so
    the HAM clock gate (4/8 -> 8/8 after ~3.4 us of sustained PE
    activity) opens just as the real stream starts
Measured: 34.6-35.5 us NEFF exec (vs 42.5-43.3 us fp32r baseline),
of which ~7.2 us fixed preamble + ~3 us fixed epilogue; rel err 2.9e-3.
"""

import contextlib
import ctypes
import os
import sys
import types

import ml_dtypes
import numpy as np

BF16 = np.dtype(ml_dtypes.bfloat16)

sys.path.insert(0, "/opt/trn_rl_repo")


def _ensure_ntff_hook():
    """Provide antenv.axon_hooks if the image lacks it.

    concourse.bass_utils imports antenv.axon_hooks.get_axon_ntff_profile_hook
    when trace=True under axon; some agent images don't ship that module, in
    which case the boot's NTFF hook registration silently degrades and the
    import in bass_utils crashes. Recreate the slim ctypes hook here
    (mirrors trn_agent_boot.trn_boot._ntff_profile_via_ctypes).
    """
    try:
        import antenv.axon_hooks  # noqa: F401

        return
    except ImportError:
        pass

    so_path = "/opt/axon/libaxon_pjrt.so"
    hook = None
    if os.path.exists(so_path):
        lib = ctypes.CDLL(so_path)
        if hasattr(lib, "axon_start_nrt_profile"):
            lib.axon_start_nrt_profile.argtypes = [
                ctypes.POINTER(ctypes.c_int64),
                ctypes.c_size_t,
            ]
            lib.axon_start_nrt_profile.restype = ctypes.c_int64
            lib.axon_stop_nrt_profile.argtypes = [ctypes.c_char_p]
            lib.axon_stop_nrt_profile.restype = ctypes.c_int64

            @contextlib.contextmanager
            def hook(output_dir, device_ids):
                import jax

                jax.devices()
                if device_ids:
                    ids = (ctypes.c_int64 * len(device_ids))(*device_ids)
                    rc = lib.axon_start_nrt_profile(ids, len(device_ids))
                else:
                    rc = lib.axon_start_nrt_profile(None, 0)
                if rc != 0:
                    raise RuntimeError(f"axon_start_nrt_profile rc={rc}")
                try:
                    yield
                finally:
                    n = lib.axon_stop_nrt_profile(str(output_dir).encode())
                    if n <= 0:
                        print(
                            f"ntff profile: rc={n} writing {output_dir}",
                            file=sys.stderr,
                        )

    mod = types.ModuleType("antenv.axon_hooks")
    _state = {"hook": hook}
    mod.set_axon_ntff_profile_hook = lambda h: _state.__setitem__("hook", h)
    mod.get_axon_ntff_profile_hook = lambda: _state["hook"]
    sys.modules["antenv.axon_hooks"] = mod
    try:
        import antenv

        antenv.axon_hooks = mod
    except ImportError:
        pass


_ensure_ntff_hook()

import concourse.bass as bass
import concourse.bacc as bacc_mod
import concourse.mybir as mybir
import concourse.tile as tile
from concourse.bass import ts
from concourse.bass_utils import run_bass_kernel_spmd

N_CORES = 8
P = 128
N_TILE = 512  # one fp32 PSUM bank; also >=256 keeps fp32r at full rate

_nc_cache = {}
LAST_RESULTS = None  # BassKernelResults of the most recent run (for test.py)


N_WARM = 31  # HAM warm-up matmuls issued before the real stream


def _build_nc(T_use: int, D: int, O: int):
    KO = D // P
    NO = O // N_TILE
    mmdt = mybir.dt.bfloat16
    f32 = mybir.dt.float32

    # All matmul m-tiles are a full 128 wide: a short (e.g. 32-row)
    # remainder tile in the k-lockstep group stalls the LDWEIGHTS
    # pipeline twice per k-step (~+210 ns/step, measured). The pad
    # columns of x are never DMA'd (junk SBUF is fine: they only feed
    # psum rows that are never read) and the pad rows of out are never
    # written; only T_use rows are real.
    T_pad = -(-T_use // P) * P
    MO = T_pad // P
    m_starts = [P * i for i in range(MO)]
    o_sizes = [min(P, T_use - s) for s in m_starts]  # DVE/out rows per tile

    nc = bacc_mod.Bacc()
    xT = nc.dram_tensor("xT", [D, T_use], mmdt, kind="ExternalInput")
    w = nc.dram_tensor("w", [D, O], mmdt, kind="ExternalInput")
    bias = nc.dram_tensor("bias", [1, O], f32, kind="ExternalInput")
    out = nc.dram_tensor("out", [T_use, O], mmdt, kind="ExternalOutput")

    xT_t = xT[:, :].rearrange("(ko p) t -> p ko t", p=P)
    w_t = w[:, :].rearrange("(ko p) o -> p ko o", p=P)

    with tile.TileContext(nc) as tc:
        with (
            tc.tile_pool(name="resident", bufs=1) as rpool,
            tc.tile_pool(name="psum", bufs=7, space="PSUM") as psum_pool,
            tc.tile_pool(name="warmps", bufs=1, space="PSUM") as warm_pool,
            tc.tile_pool(name="obuf", bufs=6) as opool,
        ):
            # HAM warm-up: the PE clock gate needs ~3.4 us of sustained
            # activity to go 4/8 -> 8/8; dummy matmuls bridge the gap
            # between the framework preamble end (~7.2 us) and the first
            # real data landing (~8.5 us). The dummy psum is never read.
            warm_sb = rpool.tile([P, 128], mmdt, tag="warm")
            nc.vector.memset(warm_sb[:], 0.0)
            warm_ps = warm_pool.tile([64, 128], f32, tag="wps")
            for i in range(N_WARM):
                nc.tensor.matmul(
                    warm_ps[:],
                    lhsT=warm_sb[:, :64],
                    rhs=warm_sb[:],
                    start=True,
                    stop=True,
                )
            # Each DMA queue sustains only ~110 GB/s (~90 on the SWDGE/
            # gpsimd queue), and wave 0 consumes x(k)+w(k,0) at
            # ~250 GB/s, so ALL THREE queues carry wave 0's pieces,
            # round-robin in k order (earliest-needed first). Wave 1's
            # w(:,1) follows on the same rotation, and the bias halves
            # ride behind (the DVE first needs bias-n0 only when wave 0
            # drains, ~10 us after the stream starts).
            bias_sb = rpool.tile([P, O], f32, tag="bias")
            x_sb = {}
            w_sb = {}
            rr = [nc.sync, nc.scalar, nc.gpsimd]
            rri = 0

            def next_eng():
                nonlocal rri
                eng = rr[rri % 3]
                rri += 1
                return eng

            def load_x(k):
                xt = rpool.tile([P, T_pad], mmdt, tag=f"x{k}", name=f"x{k}")
                next_eng().dma_start(xt[:, :T_use], xT_t[:, k, :])
                x_sb[k] = xt

            def load_w(k, n):
                wt = rpool.tile([P, N_TILE], mmdt, tag=f"w{k}_{n}", name=f"w{k}_{n}")
                next_eng().dma_start(wt[:], w_t[:, k, ts(n, N_TILE)])
                w_sb[(k, n)] = wt

            # k=0's pair gates the stream start: split each piece into
            # two half-DMAs on one queue so the second issue's ~0.65 us
            # engine cost overlaps the first half's transfer (~0.6 us
            # earlier first-data, measured ~10.0 -> ~9.4 us).
            wt = rpool.tile([P, N_TILE], mmdt, tag="w0_0", name="w0_0")
            half = N_TILE // 2
            nc.sync.dma_start(wt[:, :half], w_t[:, 0, 0:half])
            nc.sync.dma_start(wt[:, half:], w_t[:, 0, half:N_TILE])
            w_sb[(0, 0)] = wt
            xt = rpool.tile([P, T_pad], mmdt, tag="x0", name="x0")
            xh = T_use // 2
            nc.scalar.dma_start(xt[:, :xh], xT_t[:, 0, :xh])
            nc.scalar.dma_start(xt[:, xh:T_use], xT_t[:, 0, xh:])
            x_sb[0] = xt
            rri = 2  # resume the rotation at gpsimd for w(1,0)
            for k in range(1, KO):
                load_w(k, 0)
                load_x(k)
                if k >= 5:  # wave-1 weights start landing before wave 0 ends
                    load_w(k - 5, 1)
            for k in range(3, KO):
                load_w(k, 1)
            nc.sync.dma_start(
                bias_sb[:, ts(0, N_TILE)],
                bias[:, ts(0, N_TILE)].to_broadcast((P, N_TILE)),
            )
            nc.scalar.dma_start(
                bias_sb[:, ts(1, N_TILE)],
                bias[:, ts(1, N_TILE)].to_broadcast((P, N_TILE)),
            )

            # One wave per n-tile: all MO psum groups accumulate in lockstep
            # over k, so the k-th step only needs x(k)/w(k,n). Output DMAs
            # alternate sync/scalar (free once inputs land); the final
            # three spread over all queues so the tail never serializes
            # behind one ~110 GB/s queue.
            n_outs = NO * MO
            out_engs = ([nc.sync, nc.scalar] * n_outs)[:n_outs]
            out_engs[n_outs - 3 : n_outs] = [nc.gpsimd, nc.sync, nc.scalar]
            def drain(n, m, ps_):
                ot = opool.tile([P, N_TILE], mmdt, tag="ot", name=f"ot{n}_{m}")
                nc.vector.tensor_add(
                    ot[: o_sizes[m]],
                    ps_[: o_sizes[m]],
                    bias_sb[: o_sizes[m], ts(n, N_TILE)],
                )
                out_engs[n * MO + m].dma_start(
                    out[m_starts[m] : m_starts[m] + o_sizes[m], ts(n, N_TILE)],
                    ot[: o_sizes[m]],
                )

            # Wave 0 is k-lockstep (x/w arrive k-major, the PE starts
            # after the first pair). Wave 1 has everything resident, so
            # it goes k-contiguous per m-tile: each tile's psum retires
            # ~1.7 us apart and its DVE+out drain hides under the
            # remaining matmuls — only the last tile's drain is tail.
            pss = [
                psum_pool.tile([P, N_TILE], f32, tag="ps", name=f"ps0_{m}")
                for m in range(MO)
            ]
            for k in range(KO):
                for m in range(MO):
                    nc.tensor.matmul(
                        pss[m][:],
                        lhsT=x_sb[k][:, m_starts[m] : m_starts[m] + P],
                        rhs=w_sb[(k, 0)][:],
                        start=(k == 0),
                        stop=(k == KO - 1),
                    )
            for m in range(MO):
                drain(0, m, pss[m])
            for m in range(MO):
                ps_ = psum_pool.tile([P, N_TILE], f32, tag="ps", name=f"ps1_{m}")
                for k in range(KO):
                    nc.tensor.matmul(
                        ps_[:],
                        lhsT=x_sb[k][:, m_starts[m] : m_starts[m] + P],
                        rhs=w_sb[(k, 1)][:],
                        start=(k == 0),
                        stop=(k == KO - 1),
                    )
                drain(1, m, ps_)
    nc.finalize()
    return nc


def kernel(x, category_id, weight, bias):
    global LAST_RESULTS
    x = np.asarray(x)
    category_id = np.asarray(category_id)
    weight = np.ascontiguousarray(np.asarray(weight), dtype=np.float32)
    bias = np.ascontiguousarray(np.asarray(bias), dtype=np.float32)

    orig_shape = x.shape
    D = orig_shape[-1]
    C, _, O = weight.shape
    assert C == N_CORES and D % P == 0 and O % N_TILE == 0

    T = int(np.prod(orig_shape[:-1]))
    x_flat = np.ascontiguousarray(x.reshape(T, D), dtype=np.float32)
    cid = category_id.reshape(T).astype(np.int64)

    idx_per_c = [np.flatnonzero(cid == c) for c in range(C)]
    counts = [len(ix) for ix in idx_per_c]
    T_pad = max(32, -(-max(counts) // 32) * 32)  # multiple of 32 (PE col-group)

    key = (T_pad, D, O)
    if key not in _nc_cache:
        _nc_cache[key] = _build_nc(T_pad, D, O)
    nc = _nc_cache[key]

    in_maps = []
    for c in range(C):
        xcT = np.zeros((D, T_pad), dtype=BF16)
        xcT[:, : counts[c]] = x_flat[idx_per_c[c]].astype(BF16).T
        in_maps.append(
            {
                "xT": xcT,
                "w": weight[c].astype(BF16),
                "bias": bias[c : c + 1],
            }
        )

    res = run_bass_kernel_spmd(nc, in_maps, list(range(N_CORES)))
    LAST_RESULTS = res

    out_flat = np.empty((T, O), dtype=np.float32)
    for c in range(C):
        out_flat[idx_per_c[c]] = res.results[c]["out"][: counts[c]].astype(np.float32)
    return out_flat.reshape(*orig_shape[:-1], O)

